# revision 5
# baseline (speedup 1.0000x reference)
"""Trainium2 Bass kernel: 3-layer spiking NN (DSNN) forward, 50 timesteps.

Strategy (8 NeuronCores, no inter-core communication inside the time loop):
  - Layer-0 drive H0 = inputs @ W0 is constant over time -> computed once on
    each core (fp32r matmuls), transposed to [feature, batch] layout.
  - Layer-0 membrane update is pure elementwise; replicated on all cores.
  - Layer-1 output features are sharded 8x (256 per core): each core runs
    spk0 @ W1[:, shard] as 16 accumulating fp32r matmuls per step.
  - Layer 2 is linear (no reset feeds back, output = final membrane), so it
    folds out of the loop: m2(T) = sum_t c_t * (spk1(t) @ W2) with constant
    coefficients c_t.  Each core accumulates A = sum_t c_t*spk1(t) over its
    shard and does one A @ W2[shard, :] matmul at the end.
  - Host sums the 8 partial [128, 512] outputs (the unshard of the K-sharded
    final matmul).
"""

import numpy as np
from contextlib import ExitStack

import concourse.bacc as bacc
import concourse.bass as bass
import concourse.mybir as mybir
import concourse.tile as tile
from concourse import bass_utils

ALPHA = 0.9
BETA = 0.85
T = 50
B = 128            # batch
F0, F1, F3 = 1024, 2048, 512
N_CORES = 8
SH = F1 // N_CORES  # 256 layer-1 features per core
KC0 = F0 // 128     # 8 contraction chunks for H0
KC1 = F1 // 128     # 16 contraction chunks for layer-1 matmul
NS = 512            # H0 free-dim slice width

f32 = mybir.dt.float32
f32r = mybir.dt.float32r
AL = mybir.AluOpType
u32 = mybir.dt.uint32


def _coeffs():
    # m2(T) = sum_{t=1..T} c[t] * h2(t);  c (0-based step s = t-1)
    c = np.zeros(T, dtype=np.float64)
    for s in range(T):
        tau = s + 1
        c[s] = sum(BETA ** (T - t) * ALPHA ** (t - tau) for t in range(tau, T + 1))
    return c.astype(np.float32)


def _build():
    nc = bacc.Bacc("TRN2", target_bir_lowering=False, debug=False)
    d_inT = nc.dram_tensor("inT", [F0, B], f32, kind="ExternalInput")
    d_W0 = nc.dram_tensor("W0", [F0, F1], f32, kind="ExternalInput")
    d_W1s = nc.dram_tensor("W1s", [F1, SH], f32r, kind="ExternalInput")
    d_W2s = nc.dram_tensor("W2s", [SH, F3], f32, kind="ExternalInput")
    d_eye = nc.dram_tensor("EYE", [128, 128], f32, kind="ExternalInput")
    d_out = nc.dram_tensor("OUT", [B, F3], f32, kind="ExternalOutput")

    coef = _coeffs()

    with tile.TileContext(nc) as tc, ExitStack() as ctx:
        const_pool = ctx.enter_context(tc.tile_pool(name="const", bufs=1))
        state_pool = ctx.enter_context(tc.tile_pool(name="state", bufs=1))
        w0_pool = ctx.enter_context(tc.tile_pool(name="w0s", bufs=3))
        htmp_pool = ctx.enter_context(tc.tile_pool(name="htmp", bufs=2))
        out_pool = ctx.enter_context(tc.tile_pool(name="outp", bufs=1))
        psH_pool = ctx.enter_context(tc.tile_pool(name="psH", bufs=2, space="PSUM"))
        psT_pool = ctx.enter_context(tc.tile_pool(name="psT", bufs=2, space="PSUM"))
        ps1_pool = ctx.enter_context(tc.tile_pool(name="ps1", bufs=2, space="PSUM"))

        # ---- resident constants -------------------------------------------------
        inT = const_pool.tile([128, KC0 * 128], f32, tag="inT")
        for k in range(KC0):
            nc.sync.dma_start(inT[:, k * 128:(k + 1) * 128],
                              d_inT.ap()[k * 128:(k + 1) * 128, :])
        W1sb = const_pool.tile([128, KC1 * SH], f32r, tag="W1sb")
        for k in range(KC1):
            nc.sync.dma_start(W1sb[:, k * SH:(k + 1) * SH],
                              d_W1s.ap()[k * 128:(k + 1) * 128, :])
        W2sb = const_pool.tile([128, (SH // 128) * F3], f32, tag="W2sb")
        for k in range(SH // 128):
            nc.sync.dma_start(W2sb[:, k * F3:(k + 1) * F3],
                              d_W2s.ap()[k * 128:(k + 1) * 128, :])
        eye = const_pool.tile([128, 128], f32, tag="eye")
        nc.sync.dma_start(eye[:], d_eye.ap())

        zeros_big = const_pool.tile([128, F1], f32, tag="zbig")
        nc.vector.memset(zeros_big[:], 0.0)
        zeros_sm = const_pool.tile([128, SH], f32, tag="zsm")
        nc.vector.memset(zeros_sm[:], 0.0)

        # ---- H0 = inputs @ W0, stored transposed [feat, batch] ------------------
        H0T = state_pool.tile([128, KC1 * 128], f32, tag="H0T")
        for n in range(F1 // NS):
            ph = psH_pool.tile([128, NS], f32, tag="psH")
            for k in range(KC0):
                w0t = w0_pool.tile([128, NS], f32, tag="w0t")
                nc.sync.dma_start(w0t[:],
                                  d_W0.ap()[k * 128:(k + 1) * 128, n * NS:(n + 1) * NS])
                nc.tensor.matmul(ph[:],
                                 inT[:, k * 128:(k + 1) * 128],
                                 w0t[:],
                                 start=(k == 0), stop=(k == KC0 - 1))
            htmp = htmp_pool.tile([128, NS], f32, tag="htmp")
            nc.scalar.copy(htmp[:], ph[:])
            for j in range(NS // 128):
                tp = psT_pool.tile([128, 128], f32, tag="psT")
                nc.tensor.transpose(tp[:], htmp[:, j * 128:(j + 1) * 128], eye[:])
                kk = n * (NS // 128) + j
                nc.vector.tensor_copy(H0T[:, kk * 128:(kk + 1) * 128], tp[:])

        # ---- state (ping-pong pairs) -------------------------------------------
        M = [state_pool.tile([128, F1], f32, tag=f"M{i}", name=f"M{i}") for i in range(2)]
        SPK0 = [state_pool.tile([128, F1], f32r, tag=f"SPK{i}", name=f"SPK{i}") for i in range(2)]
        S1 = [state_pool.tile([128, SH], f32, tag=f"S1{i}", name=f"S1{i}") for i in range(2)]
        M1 = [state_pool.tile([128, SH], f32, tag=f"M1{i}", name=f"M1{i}") for i in range(2)]
        A = [state_pool.tile([128, SH], f32, tag=f"A{i}", name=f"A{i}") for i in range(2)]
        SPK1 = [state_pool.tile([128, SH], f32, tag=f"SPK1{i}", name=f"SPK1{i}") for i in range(2)]

        nc.vector.memset(M[1][:], 0.0)
        nc.vector.memset(S1[1][:], 0.0)
        nc.vector.memset(M1[1][:], 0.0)
        nc.vector.memset(A[1][:], 0.0)

        # ---- the 50-step recurrence --------------------------------------------
        for t in range(T):
            cu, pr = t % 2, 1 - (t % 2)
            # layer 0: m' = beta*m + H0 ; spk = m' > 1 ; reset m' where spiked
            nc.vector.scalar_tensor_tensor(M[cu][:], M[pr][:], BETA, H0T[:],
                                           AL.mult, AL.add)
            nc.vector.tensor_scalar(SPK0[cu][:], M[cu][:], 1.0, None, AL.is_gt)
            nc.vector.copy_predicated(M[cu][:], SPK0[cu][:].bitcast(u32), zeros_big[:])

            # layer 1 matmul: psum[b, shard] += spk0_chunk.T @ W1s_chunk
            ph1 = ps1_pool.tile([128, SH], f32, tag="ps1")
            for k in range(KC1):
                nc.tensor.matmul(ph1[:],
                                 SPK0[cu][:, k * 128:(k + 1) * 128],
                                 W1sb[:, k * SH:(k + 1) * SH],
                                 start=(k == 0), stop=(k == KC1 - 1))

            # layer 1 state
            nc.vector.scalar_tensor_tensor(S1[cu][:], S1[pr][:], ALPHA, ph1[:],
                                           AL.mult, AL.add)
            nc.vector.scalar_tensor_tensor(M1[cu][:], M1[pr][:], BETA, S1[cu][:],
                                           AL.mult, AL.add)
            nc.vector.tensor_scalar(SPK1[cu][:], M1[cu][:], 1.0, None, AL.is_gt)
            nc.vector.copy_predicated(M1[cu][:], SPK1[cu][:].bitcast(u32), zeros_sm[:])
            # A += c_t * spk1
            nc.vector.scalar_tensor_tensor(A[cu][:], SPK1[cu][:], float(coef[t]),
                                           A[pr][:], AL.mult, AL.add)

        # ---- final: OUT_partial = A @ W2s ---------------------------------------
        Afin = A[(T - 1) % 2]
        AT = out_pool.tile([128, (SH // 128) * 128], f32, tag="AT")
        for j in range(SH // 128):
            tp = psT_pool.tile([128, 128], f32, tag="psT")
            nc.tensor.transpose(tp[:], Afin[:, j * 128:(j + 1) * 128], eye[:])
            nc.vector.tensor_copy(AT[:, j * 128:(j + 1) * 128], tp[:])
        pout = psH_pool.tile([128, F3], f32, tag="psH")
        for j in range(SH // 128):
            nc.tensor.matmul(pout[:],
                             AT[:, j * 128:(j + 1) * 128],
                             W2sb[:, j * F3:(j + 1) * F3],
                             start=(j == 0), stop=(j == SH // 128 - 1))
        outsb = out_pool.tile([128, F3], f32, tag="outsb")
        nc.scalar.copy(outsb[:], pout[:])
        nc.sync.dma_start(d_out.ap(), outsb[:])

    nc.compile()
    return nc


_NC_CACHE = []


def _get_nc():
    if not _NC_CACHE:
        _NC_CACHE.append(_build())
    return _NC_CACHE[0]


def kernel(inputs, W0, W1, W2):
    inputs = np.asarray(inputs, dtype=np.float32)
    W0 = np.asarray(W0, dtype=np.float32)
    W1 = np.asarray(W1, dtype=np.float32)
    W2 = np.asarray(W2, dtype=np.float32)

    nc = _get_nc()
    inT = np.ascontiguousarray(inputs.T)
    eye = np.eye(128, dtype=np.float32)
    in_maps = []
    for c in range(N_CORES):
        in_maps.append({
            "inT": inT,
            "W0": W0,
            "W1s": np.ascontiguousarray(W1[:, c * SH:(c + 1) * SH]),
            "W2s": np.ascontiguousarray(W2[c * SH:(c + 1) * SH, :]),
            "EYE": eye,
        })
    res = bass_utils.run_bass_kernel_spmd(nc, in_maps, core_ids=list(range(N_CORES)))
    out = np.zeros((B, F3), dtype=np.float32)
    for c in range(N_CORES):
        out += res.results[c]["OUT"]
    return out


# revision 28
# speedup vs baseline: 10996.1693x; 10996.1693x over previous
"""Trainium2 Bass kernel: 3-layer spiking NN (DSNN) forward, 50 timesteps.

Strategy (8 NeuronCores, no inter-core communication inside the time loop):
  - Layer-0 drive H0 = inputs @ W0 is constant over time -> computed once on
    each core in exact fp32 (layer-0 spike trains are chaotically sensitive
    to H0 error; fp32r or bf16-split H0 was measured to fail), transposed to
    [feature, batch] layout via PE transposes.
  - Layer-0 membrane recurrence runs on the PRE-RESET state MP and collapses
    into one custom fused DVE op per step (registered below as
    DSNN_LIF_STEP): MP' = beta * (MP * (MP <= 1)) + H0.  Spike extraction is
    off the recurrence chain on the ACT engine: spk = Sigmoid(BIG*(MP'-1)),
    which saturates to exactly {0,1} in fp32, written as float32r for the PE.
  - Layer-1 output features are sharded 8x (256 per core): the PE computes
    psum = spk0 @ W1[:, shard] as 16 accumulating float32r matmuls per step
    (full-rate fp32; spikes are exact in fp32r, W1 rounding error measured
    ~1e-4 relative).  Layer-1 state: s1 via one DVE scalar_tensor_tensor,
    membrane via the same fused LIF op, spk1 via ACT sigmoid.
  - Layer 2 is linear (no reset feeds back; output = final membrane):
    m2(T) = sum_t c_t * (spk1(t) @ W2) with constant c_t.  The PE accumulates
    A = sum_t c_t*spk1(t) in a pinned PSUM bank (identity matmuls with
    c_t-scaled identity stationaries built on device); at the end one
    A @ W2[shard, :] fp32 matmul produces the per-core partial output.
  - Host sums the 8 partial [128, 512] outputs (the unshard of the K-sharded
    final matmul).
"""

import numpy as np
from contextlib import ExitStack

import concourse.bacc as bacc
import concourse.bass as bass
import concourse.mybir as mybir
import concourse.tile as tile
from concourse import bass_utils
from concourse import dve_ops as _DOPS
from concourse.dve_spec import Spec, Src0, Src1, C0, One, lower as _dve_lower
from concourse.dve_uop import DveOpSpec as _DveOpSpec

ALPHA = 0.9
BETA = 0.85
T = 50
B = 128            # batch
F0, F1, F3 = 1024, 2048, 512
N_CORES = 8
SH = F1 // N_CORES  # 256 layer-1 features per core
KC0 = F0 // 128     # 8 contraction chunks for H0
KC1 = F1 // 128     # 16 contraction chunks for layer-1 matmul
NS = 512            # H0 free-dim slice width
BIG = 1.0e30

f32 = mybir.dt.float32
f32r = mybir.dt.float32r
u32 = mybir.dt.uint32
AL = mybir.AluOpType
AF = mybir.ActivationFunctionType

# layer-0 slab pipelining: LIF in LIF_SLABS chunks (DVE), spike extraction
# in SIG_SLABS chunks (ACT) so the PE can start consuming early
LIF_SLABS = 2
SIG_SLABS = 4


def _register_lif():
    """Fused LIF step on the pre-reset membrane:
    out = beta * (in0 * (in0 <= 1)) + in1  (reset-gate, leak, drive)."""
    name = "DSNN_LIF_STEP"
    for op in _DOPS.OPS:
        if op.name == name:
            return op
    body = (Src0 * (Src0 <= One)) * C0 + Src1
    spec = Spec(body=body,
                reference=lambda in0, in1, s0, s1, imm2:
                    ((in0 * (in0 <= 1.0)) * np.float32(s0) + in1).astype(np.float32))
    row = max(_DOPS._SUB_OPCODE_FOR_NAME.values()) + 1
    _DOPS._SUB_OPCODE_FOR_NAME[name] = row
    shas = {}
    for ver in ("v3", "v4"):
        uops = _dve_lower(spec, ver=ver)
        shas[ver] = _DveOpSpec(name=name, opcode=row, uops=uops, rd1_en=True).sha(ver)
    op = _DOPS.DveOp(name, spec, subdim=False, uops_sha=shas)
    _DOPS.OPS.append(op)
    _DOPS.CUSTOM_DVE_SPECS[name] = spec
    return op


LIF = _register_lif()


def _coeffs():
    # m2(T) = sum_{t=1..T} c[t-1] * h2(t)
    c = np.zeros(T, dtype=np.float64)
    for s in range(T):
        tau = s + 1
        c[s] = sum(BETA ** (T - t) * ALPHA ** (t - tau) for t in range(tau, T + 1))
    return c.astype(np.float32)


def _build():
    nc = bacc.Bacc("TRN2", target_bir_lowering=False, debug=False)
    d_inT = nc.dram_tensor("inT", [F0, B], f32, kind="ExternalInput")
    d_W0 = nc.dram_tensor("W0", [F0, F1], f32, kind="ExternalInput")
    d_W1s = nc.dram_tensor("W1s", [F1, SH], f32r, kind="ExternalInput")
    d_W2s = nc.dram_tensor("W2s", [SH, F3], f32, kind="ExternalInput")
    d_eye = nc.dram_tensor("EYE", [128, 128], f32, kind="ExternalInput")
    d_out = nc.dram_tensor("OUT", [B, F3], f32, kind="ExternalOutput")

    with tile.TileContext(nc) as tc, ExitStack() as ctx:
        const_pool = ctx.enter_context(tc.tile_pool(name="const", bufs=1))
        state_pool = ctx.enter_context(tc.tile_pool(name="state", bufs=1))
        w0_pool = ctx.enter_context(tc.tile_pool(name="w0s", bufs=3))
        htmp_pool = ctx.enter_context(tc.tile_pool(name="htmp", bufs=2))
        out_pool = ctx.enter_context(tc.tile_pool(name="outp", bufs=1))
        psH_pool = ctx.enter_context(tc.tile_pool(name="psH", bufs=4, space="PSUM"))
        psT_pool = ctx.enter_context(tc.tile_pool(name="psT", bufs=1, space="PSUM"))
        ps1_pool = ctx.enter_context(tc.tile_pool(name="ps1", bufs=2, space="PSUM"))
        psA_pool = ctx.enter_context(tc.tile_pool(name="psA", bufs=1, space="PSUM"))

        # ---- resident constants ------------------------------------------------
        inT = const_pool.tile([128, KC0 * 128], f32, tag="inT")
        nc.sync.dma_start(inT[:].rearrange("p (k b) -> p k b", k=KC0),
                          d_inT.ap().rearrange("(k p) b -> p k b", p=128))
        eye = const_pool.tile([128, 128], f32, tag="eye")
        nc.sync.dma_start(eye[:], d_eye.ap())
        bnbig = const_pool.tile([128, 1], f32, tag="bnbig")
        nc.vector.memset(bnbig[:], -BIG)

        # ---- H0 = inputs @ W0 (exact fp32), stored transposed [feat, batch] ----
        H0T = state_pool.tile([128, KC1 * 128], f32, tag="H0T")
        phs = [psH_pool.tile([128, NS], f32, tag="psH", name=f"psH{n}")
               for n in range(F1 // NS)]
        for k in range(KC0):
            w0t = w0_pool.tile([128, F1], f32, tag="w0t")
            nc.sync.dma_start(w0t[:], d_W0.ap()[k * 128:(k + 1) * 128, :])
            for n in range(F1 // NS):
                nc.tensor.matmul(phs[n][:],
                                 inT[:, k * 128:(k + 1) * 128],
                                 w0t[:, n * NS:(n + 1) * NS],
                                 start=(k == 0), stop=(k == KC0 - 1))
        W1sb = const_pool.tile([128, KC1 * SH], f32r, tag="W1sb")
        nc.sync.dma_start(W1sb[:].rearrange("p (k s) -> p k s", k=KC1),
                          d_W1s.ap().rearrange("(k p) s -> p k s", p=128))
        W2sb = const_pool.tile([128, (SH // 128) * F3], f32, tag="W2sb")
        nc.sync.dma_start(W2sb[:].rearrange("p (k o) -> p k o", k=SH // 128),
                          d_W2s.ap().rearrange("(k p) o -> p k o", p=128))
        for n in range(F1 // NS):
            htmp = htmp_pool.tile([128, NS], f32, tag="htmp")
            nc.scalar.copy(htmp[:], phs[n][:])
            for j in range(NS // 128):
                tp = psT_pool.tile([128, 128], f32, tag="psT")
                nc.tensor.transpose(tp[:], htmp[:, j * 128:(j + 1) * 128], eye[:])
                kk = n * (NS // 128) + j
                nc.vector.tensor_copy(H0T[:, kk * 128:(kk + 1) * 128], tp[:])

        # c_t-scaled identity tiles for the A accumulation, built on device
        ctis = const_pool.tile([128, T * 128], f32r, tag="ctis")
        coefs = _coeffs()
        for t in range(T):
            nc.vector.tensor_scalar(ctis[:, t * 128:(t + 1) * 128], eye[:],
                                    float(coefs[t]), None, AL.mult)

        # ---- state (ping-pong pairs) -------------------------------------------
        MP = [state_pool.tile([128, F1], f32, tag=f"MP{i}", name=f"MP{i}")
              for i in range(2)]                      # layer-0 membrane (pre-reset)
        NS0 = [state_pool.tile([128, F1], f32r, tag=f"NS0{i}", name=f"NS0{i}")
               for i in range(2)]                     # spk0 in {0,1}
        S1 = [state_pool.tile([128, SH], f32, tag=f"S1{i}", name=f"S1{i}")
              for i in range(2)]
        M1P = [state_pool.tile([128, SH], f32, tag=f"M1P{i}", name=f"M1P{i}")
               for i in range(2)]                     # layer-1 membrane (pre-reset)
        NS1 = [state_pool.tile([128, SH], f32r, tag=f"NS1{i}", name=f"NS1{i}")
               for i in range(2)]

        nc.vector.memset(MP[1][:], 0.0)
        nc.vector.memset(S1[1][:], 0.0)
        nc.vector.memset(M1P[1][:], 0.0)

        A_ps = psA_pool.tile([128, SH], f32, tag="psA")

        # ---- the 50-step recurrence --------------------------------------------
        for t in range(T):
            cu, pr = t % 2, 1 - (t % 2)
            # layer 0, slab-pipelined: fused LIF step then spike extraction
            for s in range(LIF_SLABS):
                sl = slice(s * (F1 // LIF_SLABS), (s + 1) * (F1 // LIF_SLABS))
                nc.vector._custom_dve(LIF, out=MP[cu][:, sl], in0=MP[pr][:, sl],
                                      in1=H0T[:, sl], s0=BETA)
            for s in range(SIG_SLABS):
                sl = slice(s * (F1 // SIG_SLABS), (s + 1) * (F1 // SIG_SLABS))
                nc.scalar.activation(NS0[cu][:, sl], MP[cu][:, sl], AF.Sigmoid,
                                     bias=bnbig[:], scale=BIG)

            # layer 1 matmul: psum = spk0 @ W1s
            ph1 = ps1_pool.tile([128, SH], f32, tag="ps1")
            for k in range(KC1):
                nc.tensor.matmul(ph1[:],
                                 NS0[cu][:, k * 128:(k + 1) * 128],
                                 W1sb[:, k * SH:(k + 1) * SH],
                                 start=(k == 0), stop=(k == KC1 - 1))

            # layer 1 state
            nc.vector.scalar_tensor_tensor(S1[cu][:], S1[pr][:], ALPHA, ph1[:],
                                           AL.mult, AL.add)
            nc.vector._custom_dve(LIF, out=M1P[cu][:], in0=M1P[pr][:],
                                  in1=S1[cu][:], s0=BETA)
            nc.scalar.activation(NS1[cu][:], M1P[cu][:], AF.Sigmoid,
                                 bias=bnbig[:], scale=BIG)

            # A += c_t * spk1 on the PE (c_t-scaled identity as stationary)
            nc.tensor.matmul(A_ps[:], ctis[:, t * 128:(t + 1) * 128], NS1[cu][:],
                             start=(t == 0), stop=(t == T - 1), skip_group_check=True)

        # ---- final: OUT_partial = A @ W2s ---------------------------------------
        A_sb = out_pool.tile([128, SH], f32, tag="Asb")
        nc.vector.tensor_copy(A_sb[:], A_ps[:])
        AT = out_pool.tile([128, (SH // 128) * 128], f32, tag="AT")
        for j in range(SH // 128):
            tp = psT_pool.tile([128, 128], f32, tag="psT")
            nc.tensor.transpose(tp[:], A_sb[:, j * 128:(j + 1) * 128], eye[:])
            nc.vector.tensor_copy(AT[:, j * 128:(j + 1) * 128], tp[:])
        pout = psH_pool.tile([128, F3], f32, tag="psH")
        for j in range(SH // 128):
            nc.tensor.matmul(pout[:],
                             AT[:, j * 128:(j + 1) * 128],
                             W2sb[:, j * F3:(j + 1) * F3],
                             start=(j == 0), stop=(j == SH // 128 - 1))
        outsb = out_pool.tile([128, F3], f32, tag="outsb")
        nc.scalar.copy(outsb[:], pout[:])
        nc.sync.dma_start(d_out.ap(), outsb[:])

    nc.compile()
    return nc


_NC_CACHE = []


def _get_nc():
    if not _NC_CACHE:
        _NC_CACHE.append(_build())
    return _NC_CACHE[0]


def kernel(inputs, W0, W1, W2):
    inputs = np.asarray(inputs, dtype=np.float32)
    W0 = np.asarray(W0, dtype=np.float32)
    W1 = np.asarray(W1, dtype=np.float32)
    W2 = np.asarray(W2, dtype=np.float32)

    nc = _get_nc()
    inT = np.ascontiguousarray(inputs.T)
    eye = np.eye(128, dtype=np.float32)
    in_maps = []
    for c in range(N_CORES):
        in_maps.append({
            "inT": inT,
            "W0": W0,
            "W1s": np.ascontiguousarray(W1[:, c * SH:(c + 1) * SH]),
            "W2s": np.ascontiguousarray(W2[c * SH:(c + 1) * SH, :]),
            "EYE": eye,
        })
    try:
        res = bass_utils.run_bass_kernel_spmd(nc, in_maps,
                                              core_ids=list(range(N_CORES)))
    except Exception:
        res = bass_utils.run_bass_kernel_spmd(nc, in_maps,
                                              core_ids=list(range(N_CORES)))
    out = np.zeros((B, F3), dtype=np.float32)
    for c in range(N_CORES):
        out += res.results[c]["OUT"]
    return out
